# revision 17
# baseline (speedup 1.0000x reference)
"""Trainium2 Bass kernel for AdvancedMoEMixtureLoRA.

Reference computation (per token t of N = 4*2048 = 8192, D = 4096):
    z        = x @ A_w.T                       [N, 16]
    M        = 8 * (x @ M_w.T + M_b)           [N, 256] -> [N, 16, 16]
    z_mixed  = M @ z  (per token matvec)       [N, 16]
    out      = 128 * z_mixed @ B_w.T           [N, 4096]

Strategy: pure data parallel over tokens (1024 tokens per core, weights
replicated, no collectives).  Host-side prep (free, not on HW critical
path): transpose x to d-major per 128-token slab, cast everything to
bf16, fuse A_w/M_w into one [4096, 272] weight, fold all scalar factors
into the weights.

Per-core kernel, per 128-token chunk (8 chunks):
  - 1 bias matmul (K=1 ones row x M_b row) + 32 accumulating matmuls
    (stationary = x d-tile, moving = fused W [128, 272]) -> PSUM
    [128 tok, 272] = (M | z)
  - DVE mixing: P = M * broadcast(z), grouped reduce over j -> z_mixed
  - PE transpose z_mixed -> [16, 128] (pairs of chunks land in array
    row-strips 0-15 / 32-47 via tile_position col-tiling)
  - B matmuls 2-way row-packed (tile_position) against a double-strip
    B_w.T [48, 4096]; emitted interleaved into later chunks' AM loops
  - split-bank DVE/ACT parallel PSUM evacuation, 1024-col stores
"""

import sys
from collections import deque

if "/opt/trn_rl_repo" not in sys.path:
    sys.path.insert(0, "/opt/trn_rl_repo")

import ml_dtypes
import numpy as np

import concourse.bass as bass
import concourse.tile as tile
from concourse import bacc, mybir
from concourse.bass_utils import run_bass_kernel_spmd

N_CORES = 8
B, S, D = 4, 2048, 4096
N_TOK = B * S                # 8192
TPC = N_TOK // N_CORES       # tokens per core = 1024
CHUNK = 128                  # tokens per PSUM chunk
NCHUNK = TPC // CHUNK        # 8
RH = 16                      # lora rank*heads
MDIM = RH * RH               # 256
WCOLS = MDIM + RH            # 272 fused output cols (M | z)
KD = D // 128                # 32 d-chunks
OUT_D = 4096

BF = mybir.dt.bfloat16
F32 = mybir.dt.float32
NPBF = ml_dtypes.bfloat16


def build_nc():
    nc = bacc.Bacc("TRN2", target_bir_lowering=False, debug=False)
    # host-swizzled x: xsw[p, c*(KD*CHUNK) + k*CHUNK + t] = x[c*CHUNK + t, k*128 + p]
    xsw = nc.dram_tensor("xsw", [128, NCHUNK * KD * CHUNK], BF, kind="ExternalInput").ap()
    # host-swizzled W: wsw[p, k*WCOLS + m] = W.T[k*128 + p, m]
    wsw = nc.dram_tensor("wsw", [128, KD * WCOLS], BF, kind="ExternalInput").ap()
    mbr = nc.dram_tensor("mbr", [1, WCOLS], BF, kind="ExternalInput").ap()
    # double-strip B_w.T: rows 0-15 and 32-47 both hold (128*B_w).T
    bT2 = nc.dram_tensor("bT2", [48, OUT_D], BF, kind="ExternalInput").ap()
    ones = nc.dram_tensor("ones", [1, CHUNK], BF, kind="ExternalInput").ap()
    ident = nc.dram_tensor("ident", [CHUNK, CHUNK], F32, kind="ExternalInput").ap()
    out = nc.dram_tensor("out", [TPC, OUT_D], BF, kind="ExternalOutput").ap()

    SLAB = KD * CHUNK  # 4096 cols per token-slab

    with tile.TileContext(nc) as tc:
        with (
            tc.tile_pool(name="xpool", bufs=8) as xpool,
            tc.tile_pool(name="wpool", bufs=1) as wpool,
            tc.tile_pool(name="cpool", bufs=1) as cpool,
            tc.tile_pool(name="mix", bufs=2) as mixpool,
            tc.tile_pool(name="osb", bufs=4) as opool,
            tc.tile_pool(name="am", bufs=3, space="PSUM") as ampool,
            tc.tile_pool(name="bp", bufs=2, space="PSUM") as bpool,
        ):
            # All PE-critical loads share the SP HWDGE queue (strict FIFO)
            # in explicit dependency order: tiny consts, then W pieces
            # front-loaded between the first x slab halves.
            mbsb = cpool.tile([1, WCOLS], BF)
            nc.sync.dma_start(mbsb[:], mbr)
            onesb = cpool.tile([1, CHUNK], BF)
            nc.sync.dma_start(onesb[:], ones)

            wsb = wpool.tile([128, KD, WCOLS], BF)
            wflat = wsb[:].rearrange("p k m -> p (k m)")
            WQ = 4 * WCOLS

            xtiles = [
                xpool.tile([128, KD, CHUNK], BF, name=f"xs{c}", tag="xs")
                for c in range(NCHUNK)
            ]
            HS = SLAB // 2

            def load_w(q):
                nc.sync.dma_start(wflat[:, q * WQ:(q + 1) * WQ], wsw[:, q * WQ:(q + 1) * WQ])

            def load_x(c, hh):
                xsflat = xtiles[c][:].rearrange("p k t -> p (k t)")
                nc.sync.dma_start(
                    xsflat[:, hh * HS:(hh + 1) * HS],
                    xsw[:, c * SLAB + hh * HS:c * SLAB + (hh + 1) * HS],
                )

            load_w(0); load_x(0, 0); load_w(1); load_w(2); load_x(0, 1)
            load_w(3); load_w(4); load_w(5); load_w(6); load_w(7)
            for c in range(1, NCHUNK):
                load_x(c, 0); load_x(c, 1)

            bt2sb = cpool.tile([48, OUT_D], BF)
            nc.gpsimd.dma_start(bt2sb[:], bT2)
            idsb = cpool.tile([CHUNK, CHUNK], F32)
            nc.gpsimd.dma_start(idsb[:], ident)

            # B-phase emission units: ("pair", zt_sb, ca, cb, hp) or
            # ("single", zt_sb, c, hp).  Each unit = one 1024-col group.
            unitq = deque()
            osb_of = {}
            zt_of = {}  # chunk -> (zt_sb tile, strip_base)

            def emit_unit():
                if not unitq:
                    return
                kind, members, hp = unitq.popleft()
                cols = slice(hp * 1024, (hp + 1) * 1024)
                bps = []
                for (cidx, base) in members:
                    bp = bpool.tile([128, 1024], F32, name=f"bp{cidx}_{hp}", tag="bp")
                    bps.append((cidx, base, bp))
                for j in range(2):
                    csl = slice((2 * hp + j) * 512, (2 * hp + j + 1) * 512)
                    for (cidx, base, bp) in bps:
                        zt_sb, _ = zt_of[cidx]
                        nc.tensor.matmul(
                            bp[:, j * 512:(j + 1) * 512],
                            lhsT=zt_sb[base:base + RH, :],
                            rhs=bt2sb[base:base + RH, csl],
                            start=True, stop=True,
                            tile_position=(base, 0),
                        )
                for (cidx, base, bp) in bps:
                    o_sb = osb_of[cidx]
                    ob = hp * 1024
                    # split-bank parallel evacuation: DVE bank A, ACT bank B
                    nc.vector.tensor_copy(o_sb[:, ob:ob + 512], bp[:, 0:512])
                    nc.scalar.copy(o_sb[:, ob + 512:ob + 1024], bp[:, 512:1024])
                    tok = slice(cidx * CHUNK, (cidx + 1) * CHUNK)
                    nc.scalar.dma_start(out[tok, cols], o_sb[:, cols])

            def front_half(c, zm_out):
                """Fused A/M accumulation (with B units interleaved),
                DVE mixing into zm_out (a column slice of a pair tile)."""
                xs = xtiles[c]
                am = ampool.tile([128, WCOLS], F32, name=f"am{c}", tag="am")
                nc.tensor.matmul(am[:], lhsT=onesb[:], rhs=mbsb[:], start=True, stop=False)
                for k in range(KD):
                    nc.tensor.matmul(
                        am[:], lhsT=xs[:, k, :], rhs=wsb[:, k, :],
                        start=False, stop=(k == KD - 1),
                    )
                    if k % 8 == 7:
                        emit_unit()

                # z -> SBUF (scalar engine, tiny)
                z_sb = mixpool.tile([128, RH], F32, tag="z", name=f"z{c}")
                nc.scalar.copy(z_sb[:], am[:, MDIM:WCOLS])

                # P[p, i, j] = M[p, i, j] * z[p, j]
                p_sb = mixpool.tile([128, MDIM], BF, tag="p", name=f"pp{c}")
                nc.vector.tensor_mul(
                    p_sb[:].rearrange("p (i j) -> p i j", i=RH),
                    am[:, 0:MDIM].rearrange("p (i j) -> p i j", i=RH),
                    z_sb[:].unsqueeze(1).broadcast_to([128, RH, RH]),
                )
                # z_mixed[p, i] = sum_j P[p, i, j]
                nc.vector.tensor_reduce(
                    zm_out, p_sb[:].rearrange("p (i j) -> p i j", i=RH),
                    axis=mybir.AxisListType.X, op=mybir.AluOpType.add,
                )

            def transpose_pair(zmp, width, chunks):
                """One PE transpose of the [128, width] z_mixed tile ->
                [width, 128] PSUM at partition 0, evacuate to SBUF."""
                zt_ps = ampool.tile([width, CHUNK], F32, name=f"ztp{chunks[0][0]}", tag="am")
                nc.tensor.transpose(zt_ps[:], zmp[:, 0:width], idsb[:])
                zt_sb = mixpool.tile([width, CHUNK], BF, tag="zt", name=f"zt{chunks[0][0]}")
                for (cidx, base) in chunks:
                    # copy only the valid strips (rows 16-31 of a pair tile
                    # hold transposed garbage from the unused zm columns)
                    nc.scalar.copy(zt_sb[base:base + RH, :], zt_ps[base:base + RH, :])
                    zt_of[cidx] = (zt_sb, base)

            # schedule: packed pairs {0,1} {2,3} {4,5}, singles {6} {7}
            PAIRS = {1: 0, 3: 2, 5: 4}  # on finishing c, pair with PAIRS[c]
            hold = {}
            for c in range(NCHUNK):
                osb_of[c] = opool.tile([128, OUT_D], BF, name=f"osb{c}", tag="osb")
                if c in (0, 2, 4):
                    # first member of a pair: reduce into cols 0-15 of a
                    # fresh [128, 48] pair tile (cols 16-31 unused)
                    zmp = mixpool.tile([128, 48], F32, tag="zmp", name=f"zmp{c}")
                    front_half(c, zmp[:, 0:RH])
                    hold[c] = zmp
                elif c in PAIRS:
                    ca = PAIRS[c]
                    zmp = hold.pop(ca)
                    front_half(c, zmp[:, 32:32 + RH])
                    transpose_pair(zmp, 48, [(ca, 0), (c, 32)])
                    for hp in range(4):
                        unitq.append(("pair", [(ca, 0), (c, 32)], hp))
                else:
                    # singles: chunks 6 and 7
                    zmp = mixpool.tile([128, RH], F32, tag="zmp", name=f"zmp{c}")
                    front_half(c, zmp[:, 0:RH])
                    transpose_pair(zmp, RH, [(c, 0)])
                    for hp in range(4):
                        unitq.append(("single", [(c, 0)], hp))

            while unitq:
                emit_unit()

    nc.compile()
    return nc


_NC = None


def _get_nc():
    global _NC
    if _NC is None:
        _NC = build_nc()
    return _NC


def make_in_maps(x, A_w, B_w, M_w, M_b):
    x = np.asarray(x, dtype=np.float32)
    A_w = np.asarray(A_w, dtype=np.float32)
    B_w = np.asarray(B_w, dtype=np.float32)
    M_w = np.asarray(M_w, dtype=np.float32)
    M_b = np.asarray(M_b, dtype=np.float32)

    # fold scales: M' = x @ (8 M_w).T + 8 M_b ; out = z_mixed @ (128 B_w).T
    W = np.concatenate([8.0 * M_w, A_w], axis=0)              # [272, 4096]
    wT_np = W.T.astype(NPBF)                                  # [4096, 272]
    # swizzle to [128, k*272 + m] so each SBUF partition line is contiguous
    wsw_np = np.ascontiguousarray(
        wT_np.reshape(KD, 128, WCOLS).transpose(1, 0, 2).reshape(128, KD * WCOLS)
    )
    mb_np = np.concatenate([8.0 * M_b, np.zeros(RH, np.float32)]).reshape(1, WCOLS).astype(NPBF)
    bT_np = (128.0 * B_w).T.astype(NPBF)                      # [16, 4096]
    bT2_np = np.zeros((48, OUT_D), dtype=NPBF)
    bT2_np[0:RH] = bT_np
    bT2_np[32:32 + RH] = bT_np
    ones_np = np.ones((1, CHUNK), dtype=NPBF)
    id_np = np.eye(CHUNK, dtype=np.float32)

    xf = x.reshape(N_TOK, D)
    in_maps = []
    for core in range(N_CORES):
        shard = xf[core * TPC:(core + 1) * TPC].astype(NPBF)  # [1024, 4096]
        # xsw[p, c*4096 + k*128 + t] = shard[c*128 + t, k*128 + p]
        xsw_np = np.ascontiguousarray(
            shard.reshape(NCHUNK, CHUNK, KD, 128)             # [c, t, k, p]
            .transpose(3, 0, 2, 1)                            # [p, c, k, t]
            .reshape(128, NCHUNK * KD * CHUNK)
        )
        in_maps.append({
            "xsw": xsw_np, "wsw": wsw_np, "mbr": mb_np, "bT2": bT2_np,
            "ones": ones_np, "ident": id_np,
        })
    return in_maps


def assemble_out(results):
    outs = [np.asarray(results[i]["out"], dtype=np.float32) for i in range(N_CORES)]
    return np.concatenate(outs, axis=0).reshape(B, S, OUT_D)


def kernel(x, A_w, B_w, M_w, M_b):
    nc = _get_nc()
    in_maps = make_in_maps(x, A_w, B_w, M_w, M_b)
    res = run_bass_kernel_spmd(nc, in_maps, core_ids=list(range(N_CORES)))
    return assemble_out(res.results)


# revision 19
# speedup vs baseline: 1.2617x; 1.2617x over previous
"""Trainium2 Bass kernel for AdvancedMoEMixtureLoRA.

Reference computation (per token t of N = 4*2048 = 8192, D = 4096):
    z        = x @ A_w.T                       [N, 16]
    M        = 8 * (x @ M_w.T + M_b)           [N, 256] -> [N, 16, 16]
    z_mixed  = M @ z  (per token matvec)       [N, 16]
    out      = 128 * z_mixed @ B_w.T           [N, 4096]

Strategy: pure data parallel over tokens (1024 tokens per core, weights
replicated, no collectives).  Host-side prep (free, not on HW critical
path): transpose x to d-major per 128-token slab, cast everything to
bf16, fuse A_w/M_w into one [4096, 272] weight, fold all scalar factors
into the weights, build a double-strip B_w.T for row-packed B matmuls.

Per-core kernel, per 128-token chunk (8 chunks):
  - 1 bias matmul (K=1 ones row x M_b row) + 32 accumulating matmuls
    (stationary = x d-tile, moving = fused W [128, 272]) -> PSUM
    [128 tok, 272] = (M | z)
  - DVE mixing: P = M * broadcast(z), grouped reduce over j -> z_mixed
    (pairs of chunks reduce into one [128, 48] tile)
  - one PE transpose per pair -> [48, 128] (strips 0-15 / 32-47)
  - B matmuls 2-way row-packed via base_partition-derived tile_position
    against the double-strip B_w.T [48, 4096]
  - split-bank DVE/ACT parallel PSUM evacuation, 2048-col stores
"""

import sys

if "/opt/trn_rl_repo" not in sys.path:
    sys.path.insert(0, "/opt/trn_rl_repo")

import ml_dtypes
import numpy as np

import concourse.bass as bass
import concourse.tile as tile
from concourse import bacc, mybir
from concourse.bass_utils import run_bass_kernel_spmd

N_CORES = 8
B, S, D = 4, 2048, 4096
N_TOK = B * S                # 8192
TPC = N_TOK // N_CORES       # tokens per core = 1024
CHUNK = 128                  # tokens per PSUM chunk
NCHUNK = TPC // CHUNK        # 8
RH = 16                      # lora rank*heads
MDIM = RH * RH               # 256
WCOLS = MDIM + RH            # 272 fused output cols (M | z)
KD = D // 128                # 32 d-chunks
OUT_D = 4096

BF = mybir.dt.bfloat16
F32 = mybir.dt.float32
NPBF = ml_dtypes.bfloat16

# B-phase grouping: pairs {0,1} {2,3} {4,5} run 2-way row-packed,
# chunks 6 and 7 run solo (keeps the kernel tail short)
PAIR_OF = {0: 1, 2: 3, 4: 5}


def build_nc(pairing=True):
    nc = bacc.Bacc("TRN2", target_bir_lowering=False, debug=False)
    # host-swizzled x: xsw[p, c*(KD*CHUNK) + k*CHUNK + t] = x[c*CHUNK + t, k*128 + p]
    xsw = nc.dram_tensor("xsw", [128, NCHUNK * KD * CHUNK], BF, kind="ExternalInput").ap()
    # host-swizzled W: wsw[p, k*WCOLS + m] = W.T[k*128 + p, m]
    wsw = nc.dram_tensor("wsw", [128, KD * WCOLS], BF, kind="ExternalInput").ap()
    mbr = nc.dram_tensor("mbr", [1, WCOLS], BF, kind="ExternalInput").ap()
    # double-strip B_w.T: rows 0-15 and 32-47 both hold (128*B_w).T
    bT2 = nc.dram_tensor("bT2", [48, OUT_D], BF, kind="ExternalInput").ap()
    ones = nc.dram_tensor("ones", [1, CHUNK], BF, kind="ExternalInput").ap()
    ident = nc.dram_tensor("ident", [CHUNK, CHUNK], F32, kind="ExternalInput").ap()
    out = nc.dram_tensor("out", [TPC, OUT_D], BF, kind="ExternalOutput").ap()

    SLAB = KD * CHUNK  # 4096 cols per token-slab

    with tile.TileContext(nc) as tc:
        with (
            tc.tile_pool(name="xpool", bufs=8) as xpool,
            tc.tile_pool(name="wpool", bufs=1) as wpool,
            tc.tile_pool(name="cpool", bufs=1) as cpool,
            tc.tile_pool(name="mix", bufs=2) as mixpool,
            tc.tile_pool(name="osb", bufs=4) as opool,
            tc.tile_pool(name="am", bufs=2, space="PSUM") as ampool,
            tc.tile_pool(name="bp", bufs=3, space="PSUM") as bpool,
        ):
            # All PE-critical loads share the SP HWDGE queue (strict FIFO)
            # in explicit dependency order: tiny consts, then W pieces
            # front-loaded between the first x slab halves.
            mbsb = cpool.tile([1, WCOLS], BF)
            nc.sync.dma_start(mbsb[:], mbr)
            onesb = cpool.tile([1, CHUNK], BF)
            nc.sync.dma_start(onesb[:], ones)

            wsb = wpool.tile([128, KD, WCOLS], BF)
            wflat = wsb[:].rearrange("p k m -> p (k m)")
            WQ = 4 * WCOLS

            xtiles = [
                xpool.tile([128, KD, CHUNK], BF, name=f"xs{c}", tag="xs")
                for c in range(NCHUNK)
            ]
            HS = SLAB // 2

            def load_w(q):
                nc.sync.dma_start(wflat[:, q * WQ:(q + 1) * WQ], wsw[:, q * WQ:(q + 1) * WQ])

            def load_x(c, hh):
                xsflat = xtiles[c][:].rearrange("p k t -> p (k t)")
                nc.sync.dma_start(
                    xsflat[:, hh * HS:(hh + 1) * HS],
                    xsw[:, c * SLAB + hh * HS:c * SLAB + (hh + 1) * HS],
                )

            load_w(0); load_x(0, 0); load_w(1); load_w(2); load_x(0, 1)
            load_w(3); load_w(4); load_w(5); load_w(6); load_w(7)
            for c in range(1, NCHUNK):
                load_x(c, 0); load_x(c, 1)

            bt2sb = cpool.tile([48, OUT_D], BF)
            nc.gpsimd.dma_start(bt2sb[:], bT2)
            idsb = cpool.tile([CHUNK, CHUNK], F32)
            nc.gpsimd.dma_start(idsb[:], ident)

            zmp_of = {}   # chunk -> (zmp tile, col offset)
            zt_of = {}    # chunk -> (zt_sb tile, strip base)
            osb_of = {}

            def front_half(c):
                """x-driven fused A/M accumulation + DVE mixing.
                The reduce writes into this chunk's column slice of its
                pair's shared [128, 48] z_mixed tile."""
                xs = xtiles[c]
                am = ampool.tile([128, WCOLS], F32, name=f"am{c}", tag="am")
                nc.tensor.matmul(am[:], lhsT=onesb[:], rhs=mbsb[:], start=True, stop=False)
                for k in range(KD):
                    nc.tensor.matmul(
                        am[:], lhsT=xs[:, k, :], rhs=wsb[:, k, :],
                        start=False, stop=(k == KD - 1),
                    )

                z_sb = mixpool.tile([128, RH], F32, tag="z", name=f"z{c}")
                nc.scalar.copy(z_sb[:], am[:, MDIM:WCOLS])

                p_sb = mixpool.tile([128, MDIM], BF, tag="p", name=f"pp{c}")
                nc.vector.tensor_mul(
                    p_sb[:].rearrange("p (i j) -> p i j", i=RH),
                    am[:, 0:MDIM].rearrange("p (i j) -> p i j", i=RH),
                    z_sb[:].unsqueeze(1).broadcast_to([128, RH, RH]),
                )
                zmp, off = zmp_of[c]
                nc.vector.tensor_reduce(
                    zmp[:, off:off + RH], p_sb[:].rearrange("p (i j) -> p i j", i=RH),
                    axis=mybir.AxisListType.X, op=mybir.AluOpType.add,
                )

            def transpose_group(chunks):
                """One PE transpose for a pair (width 48) or single (16),
                then evacuate the valid strips to SBUF."""
                zmp, _ = zmp_of[chunks[0][0]]
                width = zmp.shape[1]
                zt_ps = ampool.tile([width, CHUNK], F32, name=f"ztp{chunks[0][0]}", tag="am")
                nc.tensor.transpose(zt_ps[:], zmp[:], idsb[:])
                zt_sb = mixpool.tile([width, CHUNK], BF, tag="zt", name=f"zt{chunks[0][0]}")
                for (cidx, base) in chunks:
                    nc.scalar.copy(zt_sb[base:base + RH, :], zt_ps[base:base + RH, :])
                    zt_of[cidx] = (zt_sb, base)

            def back_group(chunks):
                """Row-packed B matmuls + split-bank evacuation + stores
                for a group of chunks sharing one zt strip tile."""
                transpose_group(chunks)
                bps = {}
                for hp in range(4):
                    for (cidx, base) in chunks:
                        bp = bpool.tile([128, 1024], F32, name=f"bp{cidx}_{hp}", tag="bp")
                        bps[cidx] = bp
                    for j in range(2):
                        csl = slice((2 * hp + j) * 512, (2 * hp + j + 1) * 512)
                        for (cidx, base) in chunks:
                            zt_sb, _ = zt_of[cidx]
                            nc.tensor.matmul(
                                bps[cidx][:, j * 512:(j + 1) * 512],
                                lhsT=zt_sb[base:base + RH, :],
                                rhs=bt2sb[base:base + RH, csl],
                                start=True, stop=True,
                            )
                    for (cidx, base) in chunks:
                        o_sb = osb_of[cidx]
                        ob = hp * 1024
                        # split-bank parallel evacuation: DVE bank A, ACT bank B
                        nc.vector.tensor_copy(o_sb[:, ob:ob + 512], bps[cidx][:, 0:512])
                        nc.scalar.copy(o_sb[:, ob + 512:ob + 1024], bps[cidx][:, 512:1024])
                    if hp % 2 == 1:
                        for (cidx, base) in chunks:
                            tok = slice(cidx * CHUNK, (cidx + 1) * CHUNK)
                            hsl = slice((hp - 1) * 1024, (hp + 1) * 1024)
                            nc.scalar.dma_start(out[tok, hsl], osb_of[cidx][:, hsl])

            # chunk -> (pair tile key, column offset, strip base)
            groups = []  # list of chunk groups in completion order
            if pairing:
                for a, b in sorted(PAIR_OF.items()):
                    groups.append([(a, 0), (b, 32)])
                groups.append([(6, 0)])
                groups.append([(7, 0)])
            else:
                groups = [[(c, 0)] for c in range(NCHUNK)]

            for g in groups:
                width = 48 if len(g) == 2 else RH
                zmp = mixpool.tile(
                    [128, width], F32, tag="zmp", name=f"zmp{g[0][0]}", bufs=len(groups)
                )
                for (cidx, base) in g:
                    zmp_of[cidx] = (zmp, 0 if base == 0 else 32)

            # emission: front halves in chunk order; each group's back
            # half emitted after the front of the chunk AFTER the group
            # completes (hides the mixing-chain latency from the PE)
            done_after = {}  # chunk index -> groups completed at that chunk
            for gi, g in enumerate(groups):
                last = max(c for (c, _) in g)
                done_after.setdefault(last, []).append(g)

            pending = []
            for c in range(NCHUNK):
                osb_of[c] = opool.tile([128, OUT_D], BF, name=f"osb{c}", tag="osb")
                front_half(c)
                for g in pending:
                    back_group(g)
                pending = done_after.get(c, [])
            for g in pending:
                back_group(g)

    nc.compile()
    return nc


_NC = None


def _get_nc():
    global _NC
    if _NC is None:
        _NC = build_nc()
    return _NC


def make_in_maps(x, A_w, B_w, M_w, M_b):
    x = np.asarray(x, dtype=np.float32)
    A_w = np.asarray(A_w, dtype=np.float32)
    B_w = np.asarray(B_w, dtype=np.float32)
    M_w = np.asarray(M_w, dtype=np.float32)
    M_b = np.asarray(M_b, dtype=np.float32)

    # fold scales: M' = x @ (8 M_w).T + 8 M_b ; out = z_mixed @ (128 B_w).T
    W = np.concatenate([8.0 * M_w, A_w], axis=0)              # [272, 4096]
    wT_np = W.T.astype(NPBF)                                  # [4096, 272]
    # swizzle to [128, k*272 + m] so each SBUF partition line is contiguous
    wsw_np = np.ascontiguousarray(
        wT_np.reshape(KD, 128, WCOLS).transpose(1, 0, 2).reshape(128, KD * WCOLS)
    )
    mb_np = np.concatenate([8.0 * M_b, np.zeros(RH, np.float32)]).reshape(1, WCOLS).astype(NPBF)
    bT_np = (128.0 * B_w).T.astype(NPBF)                      # [16, 4096]
    bT2_np = np.zeros((48, OUT_D), dtype=NPBF)
    bT2_np[0:RH] = bT_np
    bT2_np[32:32 + RH] = bT_np
    ones_np = np.ones((1, CHUNK), dtype=NPBF)
    id_np = np.eye(CHUNK, dtype=np.float32)

    xf = x.reshape(N_TOK, D)
    in_maps = []
    for core in range(N_CORES):
        shard = xf[core * TPC:(core + 1) * TPC].astype(NPBF)  # [1024, 4096]
        # xsw[p, c*4096 + k*128 + t] = shard[c*128 + t, k*128 + p]
        xsw_np = np.ascontiguousarray(
            shard.reshape(NCHUNK, CHUNK, KD, 128)             # [c, t, k, p]
            .transpose(3, 0, 2, 1)                            # [p, c, k, t]
            .reshape(128, NCHUNK * KD * CHUNK)
        )
        in_maps.append({
            "xsw": xsw_np, "wsw": wsw_np, "mbr": mb_np, "bT2": bT2_np,
            "ones": ones_np, "ident": id_np,
        })
    return in_maps


def assemble_out(results):
    outs = [np.asarray(results[i]["out"], dtype=np.float32) for i in range(N_CORES)]
    return np.concatenate(outs, axis=0).reshape(B, S, OUT_D)


def kernel(x, A_w, B_w, M_w, M_b):
    nc = _get_nc()
    in_maps = make_in_maps(x, A_w, B_w, M_w, M_b)
    res = run_bass_kernel_spmd(nc, in_maps, core_ids=list(range(N_CORES)))
    return assemble_out(res.results)
